# revision 12
# baseline (speedup 1.0000x reference)
"""Trainium2 Bass kernel for ragged masked attention-score softmax.

Problem (B=32, T=8192, H=128):
    energy[b,t] = relu(W1 @ hidden[b] + W2 @ enc[t,b] + b_attn)   (W_attn = [W1 | W2])
    scores[b,t] = v . energy[b,t]
    out[b,0,:]  = ragged-masked softmax over t < len_seq[b], zeros after.

Strategy (8 NeuronCores, data-parallel over B):
  - Rows are sorted by len desc; slot j on every core takes one row from rank
    group [8j, 8j+8).  The per-slot static position count NP_j = max len in the
    group (rounded to 128), so all cores run one shared graph while skipping
    ~half the positions (ragged lengths are known at trace time).
  - Host passes each core's rows TRANSPOSED ([H, NP_j], H on partitions) so the
    device streams contiguous tiles straight into the TensorEngine with no
    on-device transpose of the big tensor.
  - Per 512-column tile: energy = W2T.T @ encT (PE) -> relu+hproj bias
    (ScalarE, PSUM->SBUF) -> v-dot via PE (energy tile as stationary, v as
    moving operand) accumulating scores[t,1] columns into a PSUM scores tile.
  - Epilogue: masked softmax on the [128, 4, T/128] scores tile (exact max via
    gpsimd partition_all_reduce), PE transpose per slot, DMA out.
"""

from contextlib import ExitStack

import numpy as np

import concourse.bass as bass
import concourse.tile as tile
from concourse import bacc, mybir
from concourse.bass_utils import run_bass_kernel_spmd

B, T, H = 32, 8192, 128
NCORES = 8
SLOTS = B // NCORES  # 4 rows per core
NEG = -1.0e30

# knobs
COMPUTE_DTYPE = "float32"  # "float32" | "bfloat16" for enc/W2/v/energy matmul path
CHUNK = 2048  # positions per DMA (2048 f32 = 1 MiB)
MMN = 512  # matmul moving free dim (one PSUM bank in f32)


def _np_dt(name):
    if name == "bfloat16":
        import ml_dtypes

        return np.dtype(ml_dtypes.bfloat16)
    return np.dtype(np.float32)


def _my_dt(name):
    return mybir.dt.bfloat16 if name == "bfloat16" else mybir.dt.float32


def _plan(ls, t_max):
    """Assign rows to (core, slot). Returns rows[core][slot] = b, NP[slot]."""
    order = np.argsort(-np.asarray(ls), kind="stable")
    rows = [[int(order[8 * j + i]) for j in range(SLOTS)] for i in range(NCORES)]
    NP = []
    for j in range(SLOTS):
        mx = int(max(ls[int(order[8 * j + i])] for i in range(NCORES)))
        NP.append(min(((mx + 127) // 128) * 128, t_max))
    return rows, NP


def _build(nc, NP, nt_out, dt_name):
    """Emit the Tile graph. NP: per-slot position counts (mult of 128).
    nt_out: number of 128-wide t-tiles in the output (T/128)."""
    dt_c = _my_dt(dt_name)
    f32 = mybir.dt.float32
    AF = mybir.ActivationFunctionType

    encs = [
        nc.dram_tensor(f"enc{j}", [H, NP[j]], dt_c, kind="ExternalInput").ap()
        for j in range(SLOTS)
    ]
    hid = nc.dram_tensor("hid", [H, SLOTS], f32, kind="ExternalInput").ap()
    w1t = nc.dram_tensor("w1t", [H, H], f32, kind="ExternalInput").ap()
    w2t = nc.dram_tensor("w2t", [H, H], dt_c, kind="ExternalInput").ap()
    bvec = nc.dram_tensor("bvec", [H, 1], f32, kind="ExternalInput").ap()
    vvec = nc.dram_tensor("vvec", [H, 1], dt_c, kind="ExternalInput").ap()
    maskt = nc.dram_tensor("maskt", [128, SLOTS, nt_out], f32, kind="ExternalInput").ap()
    ident = nc.dram_tensor("ident", [128, 128], f32, kind="ExternalInput").ap()
    out = nc.dram_tensor("out", [SLOTS, nt_out, 128], f32, kind="ExternalOutput").ap()

    with ExitStack() as ctx:
        tc = ctx.enter_context(tile.TileContext(nc))
        singles = ctx.enter_context(tc.tile_pool(name="singles", bufs=1))
        encpool = ctx.enter_context(tc.tile_pool(name="encp", bufs=3))
        enpool = ctx.enter_context(tc.tile_pool(name="energy", bufs=3))
        smallp = ctx.enter_context(tc.tile_pool(name="small", bufs=2))
        outp = ctx.enter_context(tc.tile_pool(name="outp", bufs=2))
        ps_e = ctx.enter_context(tc.tile_pool(name="ps_e", bufs=2, space="PSUM"))
        ps_sc = ctx.enter_context(tc.tile_pool(name="ps_sc", bufs=1, space="PSUM"))
        ps_h = ctx.enter_context(tc.tile_pool(name="ps_h", bufs=1, space="PSUM"))
        ps_o = ctx.enter_context(tc.tile_pool(name="ps_o", bufs=2, space="PSUM"))

        # constants
        w1t_sb = singles.tile([H, H], f32)
        nc.sync.dma_start(w1t_sb[:], w1t[:])
        w2t_sb = singles.tile([H, H], dt_c)
        nc.sync.dma_start(w2t_sb[:], w2t[:])
        bvec_sb = singles.tile([H, 1], f32)
        nc.sync.dma_start(bvec_sb[:], bvec[:])
        vvec_sb = singles.tile([H, 1], dt_c)
        nc.sync.dma_start(vvec_sb[:], vvec[:])
        hid_sb = singles.tile([H, SLOTS], f32)
        nc.sync.dma_start(hid_sb[:], hid[:])
        maskt_sb = singles.tile([128, SLOTS, nt_out], f32)
        nc.sync.dma_start(maskt_sb[:], maskt[:])
        ident_sb = singles.tile([128, 128], f32)
        nc.sync.dma_start(ident_sb[:], ident[:])

        # hproj[h,j] = W1 @ hidden_j + b
        ph = ps_h.tile([H, SLOTS], f32, tag="ps_small")
        nc.tensor.matmul(out=ph[:], lhsT=w1t_sb[:], rhs=hid_sb[:], start=True, stop=True)
        hproj = singles.tile([H, SLOTS], f32)
        nc.scalar.activation(hproj[:], ph[:], AF.Identity, bias=bvec_sb[:])

        # scores accumulator in PSUM, preset to NEG so untouched tiles mask out
        psc = ps_sc.tile([128, SLOTS, nt_out], f32)
        neg_sb = singles.tile([128, SLOTS, nt_out], f32)
        nc.vector.memset(neg_sb[:], NEG)
        nc.vector.tensor_copy(psc[:], neg_sb[:])

        # hot loop
        for j in range(SLOTS):
            npj = NP[j]
            for c0 in range(0, npj, CHUNK):
                cw = min(CHUNK, npj - c0)
                et = encpool.tile([H, CHUNK], dt_c, tag="enc")
                nc.sync.dma_start(et[:, :cw], encs[j][:, c0 : c0 + cw])
                for s in range(0, cw, MMN):
                    sw = min(MMN, cw - s)
                    pe = ps_e.tile([H, MMN], f32, tag="pe")
                    nc.tensor.matmul(
                        out=pe[:, :sw],
                        lhsT=w2t_sb[:],
                        rhs=et[:, s : s + sw],
                        start=True,
                        stop=True,
                    )
                    en = enpool.tile([H, MMN], dt_c, tag="en")
                    nc.scalar.activation(
                        en[:, :sw], pe[:, :sw], AF.Relu, bias=hproj[:, j : j + 1]
                    )
                    for k in range(0, sw, 128):
                        kw = min(128, sw - k)
                        tidx = (c0 + s + k) // 128
                        nc.tensor.matmul(
                            out=psc[:kw, j, tidx : tidx + 1],
                            lhsT=en[:, k : k + kw],
                            rhs=vvec_sb[:],
                            start=True,
                            stop=True,
                        )

        # masked softmax epilogue on psc [128, SLOTS, nt_out]
        ones1 = singles.tile([1, 128], f32)
        nc.vector.memset(ones1[:], 1.0)

        def _part_reduce_bcast(src_sb, op):
            """[128, SLOTS] -> per-slot partition reduction broadcast to [128, SLOTS]."""
            pt = ps_h.tile([SLOTS, 128], f32, tag="ps_small")
            nc.tensor.transpose(pt[:], src_sb[:], ident_sb[:])
            tsb = smallp.tile([SLOTS, 128], f32, tag="pr_tsb")
            nc.vector.tensor_copy(tsb[:], pt[:])
            red = smallp.tile([SLOTS, 1], f32, tag="pr_red")
            nc.vector.tensor_reduce(red[:], tsb[:], axis=mybir.AxisListType.X, op=op)
            if op == mybir.AluOpType.add:
                nc.vector.reciprocal(red[:], red[:])
            pr = ps_h.tile([1, SLOTS], f32, tag="ps_small")
            nc.tensor.transpose(pr[:], red[:], ident_sb[:SLOTS, :SLOTS])
            rsb = smallp.tile([1, SLOTS], f32, tag="pr_rsb")
            nc.vector.tensor_copy(rsb[:], pr[:])
            pb = ps_h.tile([128, SLOTS], f32, tag="ps_small")
            nc.tensor.matmul(out=pb[:], lhsT=ones1[:], rhs=rsb[:], start=True, stop=True)
            bsb = smallp.tile([128, SLOTS], f32, tag="pr_bsb")
            nc.vector.tensor_copy(bsb[:], pb[:])
            return bsb

        fmax = smallp.tile([128, SLOTS], f32)
        nc.vector.reduce_max(fmax[:], psc[:], axis=mybir.AxisListType.X)
        gmax = _part_reduce_bcast(fmax, mybir.AluOpType.max)
        ssh = smallp.tile([128, SLOTS, nt_out], f32, tag="ssh")
        nc.vector.tensor_sub(
            ssh[:], psc[:], gmax[:, :, None].broadcast_to([128, SLOTS, nt_out])
        )
        expm = smallp.tile([128, SLOTS, nt_out], f32, tag="expm")
        nc.scalar.activation(expm[:], ssh[:], AF.Exp)
        nc.vector.tensor_mul(expm[:], expm[:], maskt_sb[:])
        fsum = smallp.tile([128, SLOTS], f32)
        nc.vector.reduce_sum(fsum[:], expm[:], axis=mybir.AxisListType.X)
        grec = _part_reduce_bcast(fsum, mybir.AluOpType.add)
        attn = smallp.tile([128, SLOTS, nt_out], f32, tag="attn")
        nc.vector.tensor_mul(
            attn[:], expm[:], grec[:, :, None].broadcast_to([128, SLOTS, nt_out])
        )

        # transpose each slot to t-major and store
        for j in range(SLOTS):
            po = ps_o.tile([nt_out, 128], f32, tag="po")
            nc.tensor.transpose(po[:], attn[:, j, :], ident_sb[:])
            ob = outp.tile([nt_out, 128], f32, tag="ob")
            nc.vector.tensor_copy(ob[:], po[:])
            nc.sync.dma_start(out[j], ob[:])


def _make_inmaps(hidden, enc, ls, W_attn, b_attn, v, rows, NP, nt_out, dt_name):
    np_c = _np_dt(dt_name)
    f32 = np.float32
    w1t = np.ascontiguousarray(W_attn[:, :H].T).astype(f32)
    w2t = np.ascontiguousarray(W_attn[:, H:].T).astype(np_c)
    bvec = np.ascontiguousarray(b_attn.reshape(H, 1)).astype(f32)
    vvec = np.ascontiguousarray(v.reshape(H, 1)).astype(np_c)
    ident = np.eye(128, dtype=f32)
    tgrid = (np.arange(nt_out)[None, :] * 128 + np.arange(128)[:, None])  # [128, nt]

    in_maps = []
    for i in range(NCORES):
        m = {"w1t": w1t, "w2t": w2t, "bvec": bvec, "vvec": vvec, "ident": ident}
        hid = np.empty((H, SLOTS), f32)
        maskt = np.empty((128, SLOTS, nt_out), f32)
        for j in range(SLOTS):
            b = rows[i][j]
            m[f"enc{j}"] = np.ascontiguousarray(enc[: NP[j], b, :].T).astype(np_c)
            hid[:, j] = hidden[b, :]
            maskt[:, j, :] = (tgrid < int(ls[b])).astype(f32)
        m["hid"] = hid
        m["maskt"] = maskt
        in_maps.append(m)
    return in_maps


def run(inputs, trace=False, **spmd_kwargs):
    hidden = np.asarray(inputs["hidden"], dtype=np.float32)
    enc = np.asarray(inputs["encoder_outputs"], dtype=np.float32)
    ls = np.asarray(inputs["len_seq"]).astype(np.int64)
    W_attn = np.asarray(inputs["W_attn"], dtype=np.float32)
    b_attn = np.asarray(inputs["b_attn"], dtype=np.float32)
    v = np.asarray(inputs["v"], dtype=np.float32)
    t_len = enc.shape[0]
    nt_out = t_len // 128

    rows, NP = _plan(ls, t_len)
    nc = bacc.Bacc("TRN2", target_bir_lowering=False, debug=False)
    _build(nc, NP, nt_out, COMPUTE_DTYPE)
    nc.compile()
    in_maps = _make_inmaps(hidden, enc, ls, W_attn, b_attn, v, rows, NP, nt_out,
                           COMPUTE_DTYPE)
    res = run_bass_kernel_spmd(
        nc, in_maps, core_ids=list(range(NCORES)), trace=trace, **spmd_kwargs
    )

    final = np.zeros((B, 1, t_len), dtype=np.float32)
    for i in range(NCORES):
        o = np.asarray(res.results[i]["out"], dtype=np.float32).reshape(SLOTS, t_len)
        for j in range(SLOTS):
            b = rows[i][j]
            ln = int(ls[b])
            final[b, 0, :ln] = o[j, :ln]
    return final, res


def kernel(**inputs):
    final, _ = run(inputs, trace=False)
    return final


# revision 15
# speedup vs baseline: 1.6401x; 1.6401x over previous
"""Trainium2 Bass kernel for ragged masked attention-score softmax.

Problem (B=32, T=8192, H=128):
    energy[b,t] = relu(W1 @ hidden[b] + W2 @ enc[t,b] + b_attn)   (W_attn = [W1 | W2])
    scores[b,t] = v . energy[b,t]
    out[b,0,:]  = ragged-masked softmax over t < len_seq[b], zeros after.

Strategy (8 NeuronCores, data-parallel over B):
  - Rows are sorted by len desc; slot j on every core takes one row from rank
    group [8j, 8j+8).  The per-slot static position count NP_j = max len in the
    group (rounded to 128), so all cores run one shared graph while skipping
    ~half the positions (ragged lengths are known at trace time).
  - Host passes each core's rows TRANSPOSED ([H, NP_j], H on partitions) so the
    device streams contiguous tiles straight into the TensorEngine with no
    on-device transpose of the big tensor.
  - Per 512-column tile: energy = W2T.T @ encT (PE) -> relu+hproj bias
    (ScalarE, PSUM->SBUF) -> v-dot via PE (energy tile as stationary, v as
    moving operand) accumulating scores[t,1] columns into a PSUM scores tile.
  - Epilogue: masked softmax on the [128, 4, T/128] scores tile (exact max via
    gpsimd partition_all_reduce), PE transpose per slot, DMA out.
"""

from contextlib import ExitStack

import numpy as np

import concourse.bass as bass
import concourse.tile as tile
from concourse import bacc, mybir
from concourse.bass_utils import run_bass_kernel_spmd

B, T, H = 32, 8192, 128
NCORES = 8
SLOTS = B // NCORES  # 4 rows per core
NEG = -1.0e30

# knobs
COMPUTE_DTYPE = "bfloat16"  # "float32" | "bfloat16" for enc/W2/v/energy matmul path
CHUNK = 2048  # positions per DMA (2048 f32 = 1 MiB)
MMN = 512  # matmul moving free dim (one PSUM bank in f32)


def _np_dt(name):
    if name == "bfloat16":
        import ml_dtypes

        return np.dtype(ml_dtypes.bfloat16)
    return np.dtype(np.float32)


def _my_dt(name):
    return mybir.dt.bfloat16 if name == "bfloat16" else mybir.dt.float32


def _plan(ls, t_max):
    """Assign rows to (core, slot). Returns rows[core][slot] = b, NP[slot]."""
    order = np.argsort(-np.asarray(ls), kind="stable")
    rows = [[int(order[8 * j + i]) for j in range(SLOTS)] for i in range(NCORES)]
    NP = []
    for j in range(SLOTS):
        mx = int(max(ls[int(order[8 * j + i])] for i in range(NCORES)))
        NP.append(min(((mx + 127) // 128) * 128, t_max))
    return rows, NP


def _build(nc, NP, nt_out, dt_name):
    """Emit the Tile graph. NP: per-slot position counts (mult of 128).
    nt_out: number of 128-wide t-tiles in the output (T/128)."""
    dt_c = _my_dt(dt_name)
    f32 = mybir.dt.float32
    AF = mybir.ActivationFunctionType

    encs = [
        nc.dram_tensor(f"enc{j}", [H, NP[j]], dt_c, kind="ExternalInput").ap()
        for j in range(SLOTS)
    ]
    hid = nc.dram_tensor("hid", [H, SLOTS], f32, kind="ExternalInput").ap()
    w1t = nc.dram_tensor("w1t", [H, H], f32, kind="ExternalInput").ap()
    w2t = nc.dram_tensor("w2t", [H, H], dt_c, kind="ExternalInput").ap()
    bvec = nc.dram_tensor("bvec", [H, 1], f32, kind="ExternalInput").ap()
    vvec = nc.dram_tensor("vvec", [H, 1], dt_c, kind="ExternalInput").ap()
    maskt = nc.dram_tensor("maskt", [128, SLOTS, nt_out], f32, kind="ExternalInput").ap()
    ident = nc.dram_tensor("ident", [128, 128], f32, kind="ExternalInput").ap()
    out = nc.dram_tensor("out", [SLOTS, nt_out, 128], f32, kind="ExternalOutput").ap()

    with ExitStack() as ctx:
        tc = ctx.enter_context(tile.TileContext(nc))
        singles = ctx.enter_context(tc.tile_pool(name="singles", bufs=1))
        encpool = ctx.enter_context(tc.tile_pool(name="encp", bufs=3))
        enpool = ctx.enter_context(tc.tile_pool(name="energy", bufs=3))
        smallp = ctx.enter_context(tc.tile_pool(name="small", bufs=2))
        outp = ctx.enter_context(tc.tile_pool(name="outp", bufs=2))
        ps_e = ctx.enter_context(tc.tile_pool(name="ps_e", bufs=2, space="PSUM"))
        ps_sc = ctx.enter_context(tc.tile_pool(name="ps_sc", bufs=2, space="PSUM"))
        ps_h = ctx.enter_context(tc.tile_pool(name="ps_h", bufs=2, space="PSUM"))
        ps_o = ctx.enter_context(tc.tile_pool(name="ps_o", bufs=2, space="PSUM"))

        # constants
        w1t_sb = singles.tile([H, H], f32)
        nc.sync.dma_start(w1t_sb[:], w1t[:])
        w2t_sb = singles.tile([H, H], dt_c)
        nc.sync.dma_start(w2t_sb[:], w2t[:])
        bvec_sb = singles.tile([H, 1], f32)
        nc.sync.dma_start(bvec_sb[:], bvec[:])
        vvec_sb = singles.tile([H, 1], dt_c)
        nc.sync.dma_start(vvec_sb[:], vvec[:])
        hid_sb = singles.tile([H, SLOTS], f32)
        nc.sync.dma_start(hid_sb[:], hid[:])
        maskt_sb = singles.tile([128, SLOTS, nt_out], f32)
        nc.sync.dma_start(maskt_sb[:], maskt[:])
        ident_sb = singles.tile([128, 128], f32)
        nc.sync.dma_start(ident_sb[:], ident[:])

        ones1 = singles.tile([1, 128], f32)
        nc.vector.memset(ones1[:], 1.0)
        ones_col = singles.tile([128, 1], f32)
        nc.vector.memset(ones_col[:], 1.0)

        # hproj[h,j] = W1 @ hidden_j + b
        ph = ps_h.tile([H, SLOTS], f32, tag="ps_small")
        nc.tensor.matmul(out=ph[:], lhsT=w1t_sb[:], rhs=hid_sb[:], start=True, stop=True)
        hproj = singles.tile([H, SLOTS], f32)
        nc.scalar.activation(hproj[:], ph[:], AF.Identity, bias=bvec_sb[:])

        relu_ctr = 0
        for j in range(SLOTS):
            npj = NP[j]
            nv = npj // 128  # valid score tile-columns for this slot
            psc = ps_sc.tile([128, nt_out], f32, tag="psc")

            # ---- hot loop for slot j
            for c0 in range(0, npj, CHUNK):
                cw = min(CHUNK, npj - c0)
                et = encpool.tile([H, CHUNK], dt_c, tag="enc")
                nc.sync.dma_start(et[:, :cw], encs[j][:, c0 : c0 + cw])
                for s in range(0, cw, MMN):
                    sw = min(MMN, cw - s)
                    pe = ps_e.tile([H, MMN], f32, tag="pe")
                    nc.tensor.matmul(
                        out=pe[:, :sw],
                        lhsT=w2t_sb[:],
                        rhs=et[:, s : s + sw],
                        start=True,
                        stop=True,
                    )
                    en = enpool.tile([H, MMN], dt_c, tag="en")
                    # split bias+relu across ScalarE and VectorE
                    if relu_ctr % 2 == 0:
                        nc.scalar.activation(
                            en[:, :sw], pe[:, :sw], AF.Relu, bias=hproj[:, j : j + 1]
                        )
                    else:
                        nc.vector.tensor_scalar(
                            out=en[:, :sw],
                            in0=pe[:, :sw],
                            scalar1=hproj[:, j : j + 1],
                            scalar2=0.0,
                            op0=mybir.AluOpType.add,
                            op1=mybir.AluOpType.max,
                        )
                    relu_ctr += 1
                    for k in range(0, sw, 128):
                        kw = min(128, sw - k)
                        tidx = (c0 + s + k) // 128
                        nc.tensor.matmul(
                            out=psc[:kw, tidx : tidx + 1],
                            lhsT=en[:, k : k + kw],
                            rhs=vvec_sb[:],
                            start=True,
                            stop=True,
                        )

            # ---- fused masked softmax for slot j (overlaps next slot's loop)
            fmax = smallp.tile([128, 1], f32, tag="fmax")
            nc.vector.reduce_max(fmax[:], psc[:, :nv], axis=mybir.AxisListType.X)
            pmt = ps_h.tile([1, 128], f32, tag="ps_small")
            nc.tensor.transpose(pmt[:], fmax[:], ident_sb[:])
            mrow = smallp.tile([1, 128], f32, tag="mrow")
            nc.vector.tensor_copy(mrow[:], pmt[:])
            negm = smallp.tile([1, 1], f32, tag="negm")
            nc.vector.tensor_reduce(
                negm[:], mrow[:], axis=mybir.AxisListType.X,
                op=mybir.AluOpType.max, negate=True,
            )
            pnb = ps_h.tile([128, 1], f32, tag="ps_small")
            nc.tensor.matmul(out=pnb[:], lhsT=ones1[:], rhs=negm[:], start=True, stop=True)
            negmb = smallp.tile([128, 1], f32, tag="negmb")
            nc.vector.tensor_copy(negmb[:], pnb[:])
            expm = smallp.tile([128, nt_out], f32, tag="expm")
            nc.scalar.activation(expm[:, :nv], psc[:, :nv], AF.Exp, bias=negmb[:])
            nc.vector.tensor_mul(expm[:, :nv], expm[:, :nv], maskt_sb[:, j, :nv])
            psr = ps_h.tile([1, nt_out], f32, tag="ps_small")
            nc.tensor.matmul(
                out=psr[:, :nv], lhsT=ones_col[:], rhs=expm[:, :nv], start=True, stop=True
            )
            srow = smallp.tile([1, nt_out], f32, tag="srow")
            nc.vector.tensor_copy(srow[:, :nv], psr[:, :nv])
            s11 = smallp.tile([1, 1], f32, tag="s11")
            nc.vector.reduce_sum(s11[:], srow[:, :nv], axis=mybir.AxisListType.X)
            nc.vector.reciprocal(s11[:], s11[:])
            prb = ps_h.tile([128, 1], f32, tag="ps_small")
            nc.tensor.matmul(out=prb[:], lhsT=ones1[:], rhs=s11[:], start=True, stop=True)
            recb = smallp.tile([128, 1], f32, tag="recb")
            nc.vector.tensor_copy(recb[:], prb[:])
            attn = smallp.tile([128, nt_out], f32, tag="attn")
            nc.vector.tensor_scalar_mul(attn[:, :nv], expm[:, :nv], recb[:])

            po = ps_o.tile([nt_out, 128], f32, tag="po")
            nc.tensor.transpose(po[:nv, :], attn[:, :nv], ident_sb[:])
            ob = outp.tile([nt_out, 128], f32, tag="ob")
            nc.vector.tensor_copy(ob[:nv, :], po[:nv, :])
            nc.sync.dma_start(out[j, :nv], ob[:nv, :])


def _make_inmaps(hidden, enc, ls, W_attn, b_attn, v, rows, NP, nt_out, dt_name):
    np_c = _np_dt(dt_name)
    f32 = np.float32
    w1t = np.ascontiguousarray(W_attn[:, :H].T).astype(f32)
    w2t = np.ascontiguousarray(W_attn[:, H:].T).astype(np_c)
    bvec = np.ascontiguousarray(b_attn.reshape(H, 1)).astype(f32)
    vvec = np.ascontiguousarray(v.reshape(H, 1)).astype(np_c)
    ident = np.eye(128, dtype=f32)
    tgrid = (np.arange(nt_out)[None, :] * 128 + np.arange(128)[:, None])  # [128, nt]

    in_maps = []
    for i in range(NCORES):
        m = {"w1t": w1t, "w2t": w2t, "bvec": bvec, "vvec": vvec, "ident": ident}
        hid = np.empty((H, SLOTS), f32)
        maskt = np.empty((128, SLOTS, nt_out), f32)
        for j in range(SLOTS):
            b = rows[i][j]
            m[f"enc{j}"] = np.ascontiguousarray(enc[: NP[j], b, :].T).astype(np_c)
            hid[:, j] = hidden[b, :]
            maskt[:, j, :] = (tgrid < int(ls[b])).astype(f32)
        m["hid"] = hid
        m["maskt"] = maskt
        in_maps.append(m)
    return in_maps


def run(inputs, trace=False, **spmd_kwargs):
    hidden = np.asarray(inputs["hidden"], dtype=np.float32)
    enc = np.asarray(inputs["encoder_outputs"], dtype=np.float32)
    ls = np.asarray(inputs["len_seq"]).astype(np.int64)
    W_attn = np.asarray(inputs["W_attn"], dtype=np.float32)
    b_attn = np.asarray(inputs["b_attn"], dtype=np.float32)
    v = np.asarray(inputs["v"], dtype=np.float32)
    t_len = enc.shape[0]
    nt_out = t_len // 128

    rows, NP = _plan(ls, t_len)
    nc = bacc.Bacc("TRN2", target_bir_lowering=False, debug=False)
    _build(nc, NP, nt_out, COMPUTE_DTYPE)
    nc.compile()
    in_maps = _make_inmaps(hidden, enc, ls, W_attn, b_attn, v, rows, NP, nt_out,
                           COMPUTE_DTYPE)
    res = run_bass_kernel_spmd(
        nc, in_maps, core_ids=list(range(NCORES)), trace=trace, **spmd_kwargs
    )

    final = np.zeros((B, 1, t_len), dtype=np.float32)
    for i in range(NCORES):
        o = np.asarray(res.results[i]["out"], dtype=np.float32).reshape(SLOTS, t_len)
        for j in range(SLOTS):
            b = rows[i][j]
            ln = int(ls[b])
            final[b, 0, :ln] = o[j, :ln]
    return final, res


def kernel(**inputs):
    final, _ = run(inputs, trace=False)
    return final
